# revision 3
# baseline (speedup 1.0000x reference)
# Trainium2 Bass kernel for FJSP actor head (gnn_message_passing).
#
# Math (per batch b):
#   job_emb = ops_emb[b, next_op[b], :]                  [50, 128]  (gather)
#   u_j = job_emb @ W1[:128]   v_m = ma_emb[b] @ W1[128:]
#   h1[j,m] = relu(u_j + v_m + b1)            -> 2000 pairs (+1 noop)
#   h2 = relu(h1 @ W2 + b2);  logit = h2 @ W3 + b3
#
# Device strategy (pure data parallel over batch, 32 batches/core):
#   * ops/ma passed to the device pre-cast to bf16; all transposes are
#     done by the DMA xbar (dma_start_transpose) -- no PE transposes.
#   * Pairwise broadcast u_j + v_m is ONE matmul per 512-col chunk:
#     lhsT = jvp (rows 0-49 u, 64-103 v) [104, 128], rhs = constant 0/1
#     selection matrix smat.  b1 is folded into the relu1 bias, so no
#     b1/dummy rows; the noop logit (col 0) is computed on host.
#   * 3-stage software pipeline per batch on the PE queue:
#       S x4 (b) | W2 x4 (b-1) | W3 x4 (b-2, col-tiled -> concurrent)
#     with relu1 (2x1024-col) / relu2 (4x512-col) evacuations balanced
#     across the vector+scalar engines (the true bottleneck).
#   * Queue discipline: gathers alone on gpsimd; all other DMAs on
#     sync (+gpsimd after gathers); scalar/vector queues carry only
#     compute ops.
#   * PSUM: h1 2x[128,1024] + h2 3x[128,512] + shared(pj/lg) 1 = 8 banks.

import numpy as np
from contextlib import ExitStack

import concourse.bass as bass
import concourse.mybir as mybir
import concourse.tile as tile
from concourse import bacc
from concourse.bass_utils import run_bass_kernel_spmd

BS, N_OPS, N_JOBS, N_MA, E, H = 256, 2000, 50, 40, 128, 128
NCORES = 8
BPC = BS // NCORES            # 32 batches per core
NPAIR = N_JOBS * N_MA + 1     # 2001 logits per batch (col 0 = noop)
NPAD = 2048                   # padded logit row (cols 2001:2048 are junk)
PB = 64                       # gather rows reserved per batch (50 real + 14 pad)
NCHUNK = BPC * PB // 128      # 16 gather chunks of 128 rows
R_V0 = 64                     # v_m rows 64..103  (u_j rows at 0..49)
KJV = 104                     # S matrix rows
N3 = 472                      # trimmed width of the last 512-col chunk
CI_N = (512, 512, 512, N3)    # per-ci matmul widths
CI_O = (0, 512, 1024, 1536)   # per-ci column offsets

f32 = mybir.dt.float32
bf16 = mybir.dt.bfloat16

Relu = mybir.ActivationFunctionType.Relu
Add = mybir.AluOpType.add
Max = mybir.AluOpType.max


def _build_smat() -> np.ndarray:
    S = np.zeros((KJV, NPAD), np.float32)
    for j in range(N_JOBS):
        S[j, 1 + j * N_MA: 1 + (j + 1) * N_MA] = 1.0
    for m in range(N_MA):
        S[R_V0 + m, 1 + m: NPAIR: N_MA] = 1.0
    return S


def _build_module() -> bass.Bass:
    nc = bacc.Bacc("TRN2", target_bir_lowering=False, debug=False)
    ops = nc.dram_tensor("ops", [BPC * N_OPS, E], bf16, kind="ExternalInput")
    ma = nc.dram_tensor("ma", [BPC * N_MA, E], bf16, kind="ExternalInput")
    idx = nc.dram_tensor("idx", [128, NCHUNK], mybir.dt.int32, kind="ExternalInput")
    smat = nc.dram_tensor("smat", [KJV, NPAD], bf16, kind="ExternalInput")
    wj = nc.dram_tensor("wj", [E, H], bf16, kind="ExternalInput")
    wm = nc.dram_tensor("wm", [E, H], bf16, kind="ExternalInput")
    w2 = nc.dram_tensor("w2", [H, H], bf16, kind="ExternalInput")
    w3 = nc.dram_tensor("w3", [H, 1], bf16, kind="ExternalInput")
    b1v = nc.dram_tensor("b1v", [H], f32, kind="ExternalInput")
    b2v = nc.dram_tensor("b2v", [H], f32, kind="ExternalInput")
    out = nc.dram_tensor("out", [BPC, NPAD], f32, kind="ExternalOutput")

    with tile.TileContext(nc) as tc, ExitStack() as ctx:
        singles = ctx.enter_context(tc.tile_pool(name="singles", bufs=1))

        # ---- preamble loads: idx first (gathers wait on it) ----
        idx_s = singles.tile([128, NCHUNK], mybir.dt.int32)
        nc.sync.dma_start(out=idx_s[:], in_=idx[:])

        wj_s = singles.tile([128, H], bf16)
        nc.sync.dma_start(out=wj_s[:], in_=wj[:])
        wm_s = singles.tile([128, H], bf16)
        nc.sync.dma_start(out=wm_s[:], in_=wm[:])

        # all of ma, transposed by the DMA xbar: maT[:, 40b+m] = ma_emb[b, m, :]
        maT = singles.tile([128, BPC * N_MA], bf16)
        nc.sync.dma_start_transpose(out=maT[:], in_=ma[:])

        smat_s = singles.tile([KJV, NPAD], bf16)
        nc.sync.dma_start(out=smat_s[:], in_=smat[:])
        b1_s = singles.tile([128, 1], f32)
        nc.sync.dma_start(out=b1_s[:], in_=b1v[:].rearrange("(p o) -> p o", o=1))
        b2_s = singles.tile([128, 1], f32)
        nc.sync.dma_start(out=b2_s[:], in_=b2v[:].rearrange("(p o) -> p o", o=1))
        w2_s = singles.tile([128, H], bf16)
        nc.sync.dma_start(out=w2_s[:], in_=w2[:])
        w3_s = singles.tile([128, 1], bf16)
        nc.sync.dma_start(out=w3_s[:], in_=w3[:])

        # ---- the 16 indirect gathers, alone on the gpsimd queue ----
        grows_pool = ctx.enter_context(tc.tile_pool(name="growsp", bufs=NCHUNK))
        grows = [grows_pool.tile([128, E], bf16, tag="grows", name=f"grows{c}")
                 for c in range(NCHUNK)]
        for c in range(NCHUNK):
            nc.gpsimd.indirect_dma_start(
                out=grows[c][:], out_offset=None, in_=ops[:],
                in_offset=bass.IndirectOffsetOnAxis(ap=idx_s[:, c:c + 1], axis=0),
            )

        # ---- pools ----
        jt_pool = ctx.enter_context(tc.tile_pool(name="jt", bufs=3))
        jv_pool = ctx.enter_context(tc.tile_pool(name="jvp", bufs=3))
        h1_ps = ctx.enter_context(tc.tile_pool(name="h1ps", bufs=2, space="PSUM"))
        mid_ps = ctx.enter_context(tc.tile_pool(name="midps", bufs=3, space="PSUM"))
        sh_ps = ctx.enter_context(tc.tile_pool(name="shps", bufs=1, space="PSUM"))
        a_pool = ctx.enter_context(tc.tile_pool(name="ap", bufs=4))
        h2_pool = ctx.enter_context(tc.tile_pool(name="h2s", bufs=10))
        st_pool = ctx.enter_context(tc.tile_pool(name="st", bufs=6))

        # PE warm-up during the initial DMA window (HAM un-throttle)
        warm = singles.tile([128, 512], bf16)
        nc.vector.memset(warm[:].bitcast(mybir.dt.uint16), 0)
        for _ in range(6):
            wp = h1_ps.tile([128, 1024], f32, tag="h1", name="warmps")
            nc.tensor.matmul(out=wp[:, 0:512], lhsT=warm[:, 0:128], rhs=warm[:],
                             start=True, stop=True)

        # per-chunk setup: xbar-transpose the gathered rows, then project
        def emit_chunk_setup(c, state):
            jT = jt_pool.tile([128, 128], bf16, tag="jt", name=f"jt{c}")
            nc.sync.dma_start_transpose(out=jT[:], in_=grows[c][:])
            pj = sh_ps.tile([128, 512], f32, tag="sh", name=f"pj{c}")
            jvp = jv_pool.tile([128, 256], bf16, tag="jv", name=f"jv{c}")
            for sub in range(2):
                nc.tensor.matmul(out=pj[0:PB, 128 * sub:128 * (sub + 1)],
                                 lhsT=jT[:, sub * PB:(sub + 1) * PB],
                                 rhs=wj_s[:], start=True, stop=True)
            for sub in range(2):
                mcol = (2 * c + sub) * N_MA
                nc.tensor.matmul(out=pj[R_V0:R_V0 + N_MA, 128 * sub:128 * (sub + 1)],
                                 lhsT=maT[:, mcol:mcol + N_MA],
                                 rhs=wm_s[:], start=True, stop=True)
            # one evacuation for the whole projection block
            if c % 2 == 0:
                nc.vector.tensor_copy(out=jvp[0:KJV, :], in_=pj[0:KJV, 0:256])
            else:
                nc.scalar.copy(out=jvp[0:KJV, :], in_=pj[0:KJV, 0:256])
            state["jvp"][c] = jvp

        state = {"jvp": {}, "A": {}, "H2": {}, "lg": {}}
        emit_chunk_setup(0, state)

        def emit_S(b):
            c, sub = b // 2, b % 2
            jvp = state["jvp"][c]
            h1a = h1_ps.tile([128, 1024], f32, tag="h1", name=f"h1a{b}")
            h1b = h1_ps.tile([128, 1024], f32, tag="h1", name=f"h1b{b}")
            Aa = a_pool.tile([128, 1024], bf16, tag="A", name=f"Aa{b}")
            Ab = a_pool.tile([128, 1024], bf16, tag="A", name=f"Ab{b}")
            lhs = jvp[0:KJV, 128 * sub:128 * (sub + 1)]
            for ci in range(2):
                nc.tensor.matmul(out=h1a[:, 512 * ci:512 * ci + CI_N[ci]],
                                 lhsT=lhs,
                                 rhs=smat_s[:, CI_O[ci]:CI_O[ci] + CI_N[ci]],
                                 start=True, stop=True)
            # relu1 with fused +b1, first 1024 cols
            if b % 2 == 0:
                nc.vector.tensor_scalar(out=Aa[:], in0=h1a[:], scalar1=b1_s[:, 0:1],
                                        scalar2=0.0, op0=Add, op1=Max)
            else:
                nc.scalar.activation(out=Aa[:], in_=h1a[:], func=Relu,
                                     bias=b1_s[:, 0:1])
            for ci in range(2, 4):
                nc.tensor.matmul(
                    out=h1b[:, 512 * (ci - 2):512 * (ci - 2) + CI_N[ci]],
                    lhsT=lhs,
                    rhs=smat_s[:, CI_O[ci]:CI_O[ci] + CI_N[ci]],
                    start=True, stop=True)
            nb = 512 + N3
            if b % 2 == 0:
                nc.scalar.activation(out=Ab[:, 0:nb], in_=h1b[:, 0:nb], func=Relu,
                                     bias=b1_s[:, 0:1])
            else:
                nc.vector.tensor_scalar(out=Ab[:, 0:nb], in0=h1b[:, 0:nb],
                                        scalar1=b1_s[:, 0:1],
                                        scalar2=0.0, op0=Add, op1=Max)
            state["A"][b] = (Aa, Ab)

        def emit_W2(b):
            Aa, Ab = state["A"][b]
            h2s = []
            for ci in range(4):
                src = Aa if ci < 2 else Ab
                off = 512 * (ci % 2)
                n = CI_N[ci]
                h2p = mid_ps.tile([128, 512], f32, tag="mid", name=f"h2p{b}_{ci}")
                nc.tensor.matmul(out=h2p[:, 0:n], lhsT=w2_s[:],
                                 rhs=src[:, off:off + n], start=True, stop=True)
                H2 = h2_pool.tile([128, 512], bf16, tag="H2", name=f"H2{b}_{ci}")
                if (b + ci) % 2 == 0:
                    nc.scalar.activation(out=H2[:, 0:n], in_=h2p[:, 0:n],
                                         func=Relu, bias=b2_s[:, 0:1])
                else:
                    nc.vector.tensor_scalar(out=H2[:, 0:n], in0=h2p[:, 0:n],
                                            scalar1=b2_s[:, 0:1],
                                            scalar2=0.0, op0=Add, op1=Max)
                h2s.append(H2)
            state["H2"][b] = h2s
            del state["A"][b]

        def emit_W3(b):
            h2s = state["H2"][b]
            lg = sh_ps.tile([128, 512], f32, tag="sh", name=f"lg{b}")
            for ci in range(4):
                nc.tensor.matmul(out=lg[32 * ci:32 * ci + 1, 0:CI_N[ci]],
                                 lhsT=w3_s[:], rhs=h2s[ci][:, 0:CI_N[ci]],
                                 start=True, stop=True,
                                 tile_position=(0, 32 * ci))
            stg = st_pool.tile([128, 512], f32, tag="st", name=f"stg{b}")
            if b % 2 == 0:
                nc.scalar.copy(out=stg[0:97, :], in_=lg[0:97, :])
            else:
                nc.vector.tensor_copy(out=stg[0:97, :], in_=lg[0:97, :])
            stg4 = stg[:].rearrange("(a b) f -> a b f", b=32)[:, 0:1, :]
            eng = nc.gpsimd if b % 2 == 0 else nc.sync
            eng.dma_start(
                out=out[b:b + 1, :].rearrange("o (a f) -> o a f", a=4),
                in_=stg4)
            del state["H2"][b]

        # ---- 3-stage software pipeline over batches ----
        for b in range(BPC + 2):
            if b < BPC:
                emit_S(b)
            if 1 <= b <= BPC:
                emit_W2(b - 1)
            if b >= 2:
                emit_W3(b - 2)
            # setup for the next chunk, one batch ahead of its use
            if b < BPC and b % 2 == 1 and (b // 2 + 1) < NCHUNK:
                emit_chunk_setup(b // 2 + 1, state)

    nc.finalize()
    return nc


_CACHE: dict = {}


def _get_module() -> bass.Bass:
    if "nc" not in _CACHE:
        _CACHE["nc"] = _build_module()
    return _CACHE["nc"]


def _np_noop(dummy, W1, b1, W2, b2, W3, b3) -> float:
    dt = np.float64
    d1 = np.maximum(np.asarray(dummy, dt) @ np.asarray(W1, dt) + np.asarray(b1, dt), 0.0)
    d2 = np.maximum(d1 @ np.asarray(W2, dt) + np.asarray(b2, dt), 0.0)
    return float((d2 @ np.asarray(W3, dt) + np.asarray(b3, dt)).reshape(-1)[0])


def _make_in_maps(inputs):
    import ml_dtypes
    bf = ml_dtypes.bfloat16

    ops_emb = np.asarray(inputs["ops_emb"], dtype=np.float32).astype(bf)
    ma_emb = np.asarray(inputs["ma_emb"], dtype=np.float32).astype(bf)
    next_op = np.asarray(inputs["next_op"])
    W1 = np.asarray(inputs["W1"], dtype=np.float32).astype(bf)
    b1 = np.ascontiguousarray(np.asarray(inputs["b1"], dtype=np.float32))
    W2 = np.ascontiguousarray(np.asarray(inputs["W2"], dtype=np.float32).astype(bf))
    b2 = np.ascontiguousarray(np.asarray(inputs["b2"], dtype=np.float32))
    W3 = np.ascontiguousarray(np.asarray(inputs["W3"], dtype=np.float32).astype(bf))
    smat = _build_smat().astype(bf)
    wj = np.ascontiguousarray(W1[:E])
    wm = np.ascontiguousarray(W1[E:])

    in_maps = []
    for core in range(NCORES):
        bsl = slice(core * BPC, (core + 1) * BPC)
        no = np.asarray(next_op[bsl], dtype=np.int64)          # [BPC, 50]
        gidx = np.zeros((BPC, PB), np.int64)
        gidx[:, :N_JOBS] = no + (np.arange(BPC, dtype=np.int64)[:, None] * N_OPS)
        idx2d = np.ascontiguousarray(
            gidx.reshape(NCHUNK, 128).T.astype(np.int32))      # [128, NCHUNK]
        in_maps.append({
            "ops": np.ascontiguousarray(ops_emb[bsl].reshape(BPC * N_OPS, E)),
            "ma": np.ascontiguousarray(ma_emb[bsl].reshape(BPC * N_MA, E)),
            "idx": idx2d,
            "smat": smat,
            "wj": wj, "wm": wm, "w2": W2, "w3": W3,
            "b1v": b1, "b2v": b2,
        })
    return in_maps


def _run(inputs, trace=False, **kw):
    action_mask = np.asarray(inputs["action_mask"])
    b3 = np.asarray(inputs["b3"], dtype=np.float32)
    noop = _np_noop(inputs["dummy"], inputs["W1"], inputs["b1"],
                    inputs["W2"], inputs["b2"], inputs["W3"], inputs["b3"])
    nc = _get_module()
    in_maps = _make_in_maps(inputs)
    res = run_bass_kernel_spmd(nc, in_maps, core_ids=list(range(NCORES)),
                               trace=trace, **kw)
    logits = np.concatenate([r["out"][:, :NPAIR] for r in res.results], axis=0)
    logits = (logits + b3.reshape(-1)[0]).astype(np.float32)
    logits[:, 0] = noop
    return (logits, action_mask), res


def kernel(**inputs):
    out, _ = _run(inputs)
    return out
